# revision 44
# baseline (speedup 1.0000x reference)
"""Masked multi-head attention on 8 Trainium2 NeuronCores.

Problem (hardcoded): x[4,2048,512] f32, mask[1,4,2048,2048] bool,
Wq/Wk/Wv[512,512] f32.  out = softmax(mask? -inf : (xWq.T)(xWk.T).T/sqrt(128)) @ (xWv.T)
per head (8 heads of dim 64), merged back to [4,2048,512] f32.

Sharding: core c handles batch b=c//2 and head-quad hg=c%2 (heads hg*4..hg*4+3).
Scores are built in "ST" layout [k_partitions, q_free] so the PV matmul needs
no transposes, softmax sums ride free as a ones-column appended to V, and the
final [q, d] layout is produced with PE transposes before normalization.
The boolean mask is applied multiplicatively after exp (notmask DMA'd as bf16
0/1, one DVE tensor_mul per head tile — all on DVE; GPSIMD measured 3x slower
per tile and poisoned the pipeline). Projections are emitted as fine-grained
tasks interleaved into the attention slots so ACT/DVE start early.

All matmul operands are bf16 with f32 PSUM accumulation; measured end-to-end
rel-err vs the f32 reference is ~4e-3.

Scheduling (FLAGS, HW-validated): the boolean mask is applied with ONE
broadcast DVE multiply per kti tile (bmask), x/w chunk DMAs are interleaved
so the first projections start as soon as chunk 0 lands (dmaorder), the
deferred projection prelude is deadline-scheduled across the whole body
instead of front-loaded (sched2), and the 8 per-slice output DMAs of each
(m,qb) group are batched into a single strided DMA (obatch).  Measured via
paired-dispatch slope (median of 120 pairs): baseline 122us -> ~100-107us
per body on the same hardware/day; ~113us was the prior session's baseline
measurement of the unflagged kernel.
"""

from collections import deque

import numpy as np
import ml_dtypes

import concourse.bass as bass
import concourse.mybir as mybir
import concourse.tile as tile
from concourse import bacc
from concourse.bass_utils import run_bass_kernel_spmd
from concourse.masks import make_identity

BF16 = mybir.dt.bfloat16
F32 = mybir.dt.float32
NPBF16 = ml_dtypes.bfloat16

B, N, C = 4, 2048, 512
H, D = 8, 64
TEMP = float((2.0 * D) ** 0.5)  # sqrt(128)
P = 128
NCORES = 8
HPC = H // 2          # 4 heads per core
DQ = HPC * D          # 256 projection cols per core
KT = N // P           # 16 k tiles
QB = N // 512         # 4 q blocks
VW = D + 1            # V width incl. ones column
AV_DEPTH = 2          # software-pipeline depth for PV matmul emission


def _build_program(repeat=1, ablate=()):
    """ablate: timing-only ablations ('nomask','halfexp','noav','unpackst')."""
    nc = bacc.Bacc(
        "TRN2",
        target_bir_lowering=False,
        debug=False,
        enable_asserts=False,
        num_devices=NCORES,
    )

    xT = nc.dram_tensor("xT", [C, N], BF16, kind="ExternalInput").ap()
    wqT = nc.dram_tensor("wqT", [C, DQ], BF16, kind="ExternalInput").ap()
    wkT = nc.dram_tensor("wkT", [C, DQ], BF16, kind="ExternalInput").ap()
    wvT = nc.dram_tensor("wvT", [C, DQ], BF16, kind="ExternalInput").ap()
    nmT = nc.dram_tensor("nmT", [N, N], BF16, kind="ExternalInput").ap()
    o = nc.dram_tensor("o", [N, DQ], F32, kind="ExternalOutput").ap()

    nm_view = nmT.rearrange("(t p) q -> p t q", p=P)  # [128, 16, 2048]

    poolsplit = "poolsplit" in ablate
    with tile.TileContext(nc) as tc:
        with (
            tc.tile_pool(name="constp", bufs=1) as constp,
            tc.tile_pool(name="xp", bufs=2 if "dbuf" in ablate else 1) as xp,
            tc.tile_pool(name="wp", bufs=2 if "dbuf" in ablate else 1) as wp,
            tc.tile_pool(name="qkvp", bufs=2 if "dbuf" in ablate else 1) as qkvp,
            tc.tile_pool(name="maskp",
                         bufs=(4 if "maskp4" in ablate else
                               3 if ("bufs2" in ablate or "slack" in ablate) else 2)) as maskp,
            tc.tile_pool(name="workp",
                         bufs=(12 if "workp12" in ablate else
                               8 if ("bufs2" in ablate or "slack" in ablate) else 6)) as workp,
            tc.tile_pool(name="outp", bufs=3) as outp,
            tc.tile_pool(name="psp",
                         bufs=((4 if "psot2" in ablate else 5) if "stsplit" in ablate else
                               2 if ("bufs2" in ablate or poolsplit) else 3),
                         space="PSUM") as psp,
            tc.tile_pool(name="psot",
                         bufs=2 if ("bufs2" in ablate or "psot2" in ablate) else 1,
                         space="PSUM") as psot,
        ):
            if poolsplit:
                pp_ctx = tc.tile_pool(name="pp", bufs=2, space="PSUM")
                pp = pp_ctx.__enter__()
            else:
                pp_ctx, pp = None, psp
            if "trpool" in ablate:
                trp_ctx = tc.tile_pool(name="trp", bufs=8, space="PSUM")
                trp = trp_ctx.__enter__()
            else:
                trp_ctx, trp = None, None
            ident = constp.tile([P, P], F32)
            make_identity(nc, ident)
            ident_bf = constp.tile([P, P], BF16)
            make_identity(nc, ident_bf)
            vext_g = None
            if "vconst" in ablate:
                # the ones columns survive across bodies (v copies only touch
                # the value slices), so memset once instead of per body
                vext_g = qkvp.tile([P, KT * HPC * VW], BF16, name="vext",
                                   tag="vextg")
                nc.gpsimd.memset(vext_g, 1.0)
            for _ in range(repeat):
                _emit_body(nc, tc, xT, wqT, wkT, wvT, nm_view, o,
                           xp, wp, qkvp, maskp, workp, outp, psp, psot, ident,
                           ident_bf, pp=pp, trp=trp, vext_g=vext_g,
                           ablate=ablate)
            if trp_ctx is not None:
                trp_ctx.__exit__(None, None, None)
            if pp_ctx is not None:
                pp_ctx.__exit__(None, None, None)

    nc.compile()
    return nc


def _emit_body(nc, tc, xT, wqT, wkT, wvT, nm_view, o,
               xp, wp, qkvp, maskp, workp, outp, psp, psot, ident,
               ident_bf, pp=None, trp=None, vext_g=None, ablate=()):
    if pp is None:
        pp = psp
    if trp is None:
        trp = pp
    gmask_n = 0
    for a in ablate:
        if a.startswith("gmask"):
            gmask_n = int(a[5:])
    # ---- load inputs ----
    if "dmaorder2" in ablate:
        # Critical-path-ordered: first halves of x chunks + wq/wk interleaved
        # (unblocks the first projections), then wv, then x second halves.
        xt = [xp.tile([P, N], BF16, name=f"xt{c}", tag=f"xt{c}")
              for c in range(4)]
        ws = {w: [wp.tile([P, DQ], BF16, name=f"w{w}{c}", tag=f"w{w}{c}")
                  for c in range(4)] for w in ("q", "k", "v")}
        wdram = {"q": wqT, "k": wkT, "v": wvT}
        H1 = N // 2
        for c in range(4):
            nc.sync.dma_start(out=xt[c][:, 0:H1], in_=xT[c * P:(c + 1) * P, 0:H1])
            nc.sync.dma_start(out=ws["q"][c], in_=wdram["q"][c * P:(c + 1) * P, :])
            nc.sync.dma_start(out=ws["k"][c], in_=wdram["k"][c * P:(c + 1) * P, :])
        for c in range(4):
            nc.sync.dma_start(out=ws["v"][c], in_=wdram["v"][c * P:(c + 1) * P, :])
        for c in range(4):
            nc.sync.dma_start(out=xt[c][:, H1:N], in_=xT[c * P:(c + 1) * P, H1:N])
    elif "dmaorder" in ablate:
        # Interleave x/wq/wk chunk DMAs so the first projection matmuls can
        # start as soon as (xt[0], wq[0]) land instead of after all of x.
        xt = [xp.tile([P, N], BF16, name=f"xt{c}", tag=f"xt{c}")
              for c in range(4)]
        ws = {w: [wp.tile([P, DQ], BF16, name=f"w{w}{c}", tag=f"w{w}{c}")
                  for c in range(4)] for w in ("q", "k", "v")}
        wdram = {"q": wqT, "k": wkT, "v": wvT}
        for c in range(4):
            nc.sync.dma_start(out=xt[c], in_=xT[c * P:(c + 1) * P, :])
            nc.sync.dma_start(out=ws["q"][c], in_=wdram["q"][c * P:(c + 1) * P, :])
            nc.sync.dma_start(out=ws["k"][c], in_=wdram["k"][c * P:(c + 1) * P, :])
        for c in range(4):
            nc.sync.dma_start(out=ws["v"][c], in_=wdram["v"][c * P:(c + 1) * P, :])
    else:
        xt = []
        for c in range(4):
            t = xp.tile([P, N], BF16, name=f"xt{c}", tag=f"xt{c}")
            eng = nc.gpsimd if ("dmaq" in ablate and c % 2) else nc.sync
            eng.dma_start(out=t, in_=xT[c * P:(c + 1) * P, :])
            xt.append(t)
        ws = {}
        for wname, wdram in (("q", wqT), ("k", wkT), ("v", wvT)):
            chunks = []
            for c in range(4):
                t = wp.tile([P, DQ], BF16, name=f"w{wname}{c}", tag=f"w{wname}{c}")
                nc.sync.dma_start(out=t, in_=wdram[c * P:(c + 1) * P, :])
                chunks.append(t)
            ws[wname] = chunks

    # ---- projections ----
    # QT/KT in [d', n] layout: partition tile m holds heads (2m, 2m+1).
    qt_sb = [qkvp.tile([P, N], BF16, name=f"qt_sb{m}", tag=f"qt{m}") for m in range(2)]
    kt_sb = [qkvp.tile([P, N], BF16, name=f"kt_sb{m}", tag=f"kt{m}") for m in range(2)]

    def qk_group(wname, m, nb):
        t = (qt_sb if wname == "q" else kt_sb)[m]
        ps = pp.tile([P, 512], F32, name="proj_ps",
                     tag="st" if pp is psp else "pp")
        for c in range(4):
            nc.tensor.matmul(
                ps,
                lhsT=ws[wname][c][:, m * P:(m + 1) * P],
                rhs=xt[c][:, nb * 512:(nb + 1) * 512],
                start=(c == 0),
                stop=(c == 3),
            )
        if "projact" in ablate:
            nc.scalar.copy(t[:, nb * 512:(nb + 1) * 512], ps)
        else:
            nc.vector.tensor_copy(t[:, nb * 512:(nb + 1) * 512], ps)

    # V in [k, d'] layout with a ones column per head: [128, kt*(4*65)]
    if vext_g is not None:
        vext = vext_g
    else:
        vext = qkvp.tile([P, KT * HPC * VW], BF16)
        nc.gpsimd.memset(vext, 1.0)

    def v_group(kti):
        ps = pp.tile([P, DQ], F32, name="v_ps",
                     tag="st" if pp is psp else "pp")
        for c in range(4):
            nc.tensor.matmul(
                ps,
                lhsT=xt[c][:, kti * P:(kti + 1) * P],
                rhs=ws["v"][c],
                start=(c == 0),
                stop=(c == 3),
            )
        dst_view = vext[:, kti * HPC * VW:(kti + 1) * HPC * VW].rearrange(
            "p (h e) -> p h e", h=HPC
        )[:, :, 0:D]
        src_view = ps.rearrange("p (h e) -> p h e", h=HPC)
        nc.vector.tensor_copy(dst_view, src_view)

    # minimal prelude: only what the first (qb0, m0) scores need right away
    qk_group("q", 0, 0)
    qk_group("k", 0, 0)
    # remaining projection work, drained one task per kt slot (deadline-safe)
    prelude = deque()
    if "sched2" in ablate:
        # deadline-scheduled: emit each projection group ~4 slots before its
        # first reader (kt cols for kti=4nb..4nb+3 of every qb; qt cols for
        # qb=nb), instead of front-loading everything into qb0/m0.
        sched = [(0, ("k", 0, 1)), (4, ("k", 0, 2)), (8, ("k", 0, 3)),
                 (11, ("q", 1, 0)), (12, ("k", 1, 0)), (15, ("k", 1, 1)),
                 (19, ("k", 1, 2)), (23, ("k", 1, 3)), (26, ("q", 0, 1)),
                 (42, ("q", 1, 1)), (56, ("q", 0, 2)), (72, ("q", 1, 2)),
                 (88, ("q", 0, 3)), (104, ("q", 1, 3))]
        for due, spec in sched:
            prelude.append((due, lambda spec=spec: qk_group(*spec)))
    else:
        for i, spec in enumerate([
                ("k", 0, 1), ("k", 0, 2), ("k", 0, 3),
                ("q", 1, 0), ("k", 1, 0), ("k", 1, 1), ("k", 1, 2), ("k", 1, 3),
                ("q", 1, 1), ("q", 1, 2), ("q", 1, 3),
                ("q", 0, 1), ("q", 0, 2), ("q", 0, 3)]):
            prelude.append((i, lambda spec=spec: qk_group(*spec)))
    vqueue = deque(lambda kti=kti: v_group(kti) for kti in range(KT))
    slot_counter = [0]

    # ---- attention (software-pipelined emission) ----
    av_queue = deque()   # deferred PV-matmul emissions
    epi_stages = deque() # deferred epilogue stages of the previous (m, qb)
    obig_by_qb = {}      # obatch2: per-qb shared output staging tile

    def emit_slot():
        """Emit one deferred AV (if the pipeline is full) and one epilogue stage."""
        slot = slot_counter[0]
        slot_counter[0] += 1
        if vqueue:
            vqueue.popleft()()
        while prelude and prelude[0][0] <= slot:
            prelude.popleft()[1]()
        av_depth = (4 if "avd4" in ablate else
                    2 if "avd2" in ablate else
                    1 if "avd1" in ablate else
                    3 if ("bufs2" in ablate or "slack" in ablate) else AV_DEPTH)
        if "epifirst" in ablate:
            if epi_stages:
                epi_stages.popleft()()
            if len(av_queue) > av_depth:
                av_queue.popleft()()
        else:
            if len(av_queue) > av_depth:
                av_queue.popleft()()
            if epi_stages:
                epi_stages.popleft()()

    def make_epilogue(ot, m, qb):
        stages = []

        if "rec1" in ablate:
            def rec_stage():
                # one approx reciprocal over the sums row; the transposes then
                # carry 1/sum through to tr[:, D] for the per-slice normalize
                nc.vector.reciprocal_approx_fast(
                    out=ot[D:VW, :], in_=ot[D:VW, :])
            yield rec_stage

        def copy_stage():
            ots = outp.tile([VW, 1024], F32, name="ots", tag="ots")
            if "otact" in ablate:
                nc.scalar.copy(ots, ot)
            elif "dmaot" in ablate:
                nc.sync.dma_start(out=ots, in_=ot)
            else:
                nc.vector.tensor_copy(ots, ot)
            stages.append(ots)  # stash for slice closures
        yield copy_stage

        obatch2 = "obatch2" in ablate
        obatch = "obatch" in ablate or obatch2
        if obatch2:
            def alloc_obig():
                # one [P, sl, head, D] tile per qb shared by both m-groups;
                # flushed with a single fully-row-contiguous DMA after m=1
                if m == 0:
                    obig_by_qb[qb] = outp.tile([P, 4, 4, D], F32,
                                               name="obig2", tag="obig2")
                stages.append(obig_by_qb[qb])
            yield alloc_obig
        elif obatch:
            def alloc_obig():
                stages.append(outp.tile([P, 4, 2, D], F32, name="obig", tag="obig"))
            yield alloc_obig

        for hl in range(2):
            for sl in range(4):
                def slice_stage(hl=hl, sl=sl):
                    ots = stages[0]
                    tr = trp.tile([P, VW], F32, name="tr",
                                  tag="st" if trp is psp else
                                  ("pp" if trp is pp else "tr"))
                    nc.tensor.transpose(
                        tr,
                        ots[:, hl * 512 + sl * P: hl * 512 + (sl + 1) * P],
                        ident[0:VW, 0:VW],
                    )
                    if "rec1" in ablate:
                        rec = tr[:, D:D + 1]
                    else:
                        rec = outp.tile([P, 1], F32, name="rec", tag="rec")
                        nc.vector.reciprocal(rec, tr[:, D:D + 1])
                    if obatch2:
                        ob = stages[1][:, sl, 2 * m + hl, :]
                    elif obatch:
                        ob = stages[1][:, sl, hl, :]
                    else:
                        ob = outp.tile([P, D], F32, name="ob", tag="ob")
                    if "normact" in ablate:
                        nc.scalar.activation(
                            ob, tr[:, 0:D],
                            mybir.ActivationFunctionType.Copy, scale=rec)
                    else:
                        nc.vector.tensor_scalar_mul(ob, tr[:, 0:D], rec)
                    if not obatch:
                        h = 2 * m + hl
                        odma_eng = nc.gpsimd if "oq" in ablate else nc.sync
                        odma_eng.dma_start(
                            out=o[qb * 512 + sl * P: qb * 512 + (sl + 1) * P,
                                  h * D:(h + 1) * D],
                            in_=ob,
                        )
                yield slice_stage

        if obatch2:
            if m == 1:
                def dma_stage():
                    dst = o[qb * 512:(qb + 1) * 512, :].rearrange(
                        "(s p) (h d) -> p s h d", p=P, h=4)
                    nc.sync.dma_start(out=dst, in_=obig_by_qb.pop(qb))
                yield dma_stage
        elif obatch:
            def dma_stage():
                dst = o[qb * 512:(qb + 1) * 512,
                        2 * m * D:(2 * m + 2) * D].rearrange(
                    "(s p) (h d) -> p s h d", p=P, h=2)
                odma_eng = nc.gpsimd if "oq" in ablate else nc.sync
                odma_eng.dma_start(out=dst, in_=stages[1])
            yield dma_stage

    for qb in range(QB):
        nm = maskp.tile([P, KT, 512], BF16, name="nm", tag="nm")
        nm_eng = nc.gpsimd if "dmaq" in ablate else nc.sync
        if "nmfine" in ablate:
            for i in range(4):
                nm_eng.dma_start(
                    out=nm[:, i * 4:(i + 1) * 4, :],
                    in_=nm_view[:, i * 4:(i + 1) * 4, qb * 512:(qb + 1) * 512])
        elif "dmaorder" in ablate:
            # split so the first kti tiles land early
            nm_eng.dma_start(out=nm[:, 0:4, :],
                             in_=nm_view[:, 0:4, qb * 512:(qb + 1) * 512])
            nm_eng.dma_start(out=nm[:, 4:KT, :],
                             in_=nm_view[:, 4:KT, qb * 512:(qb + 1) * 512])
        else:
            nm_eng.dma_start(out=nm, in_=nm_view[:, :, qb * 512:(qb + 1) * 512])
        for m in range(2):
            ot = psot.tile([VW, 1024], F32, name="ot", tag="ot")
            for kti in range(KT):
                if "stsplit" in ablate:
                    exs = []
                    for hl in range(2):
                        st_h = psp.tile([P, 512], F32, name="st", tag="st")
                        nc.tensor.matmul(
                            st_h,
                            lhsT=kt_sb[m][hl * D:(hl + 1) * D, kti * P:(kti + 1) * P],
                            rhs=qt_sb[m][hl * D:(hl + 1) * D, qb * 512:(qb + 1) * 512],
                            start=True, stop=True,
                        )
                        ex_h = workp.tile([P, 512], BF16, name="exh", tag="exh")
                        nc.scalar.activation(
                            ex_h, st_h, mybir.ActivationFunctionType.Exp,
                            scale=1.0 / TEMP)
                        nc.vector.tensor_mul(ex_h, ex_h, nm[:, kti, :])
                        exs.append(ex_h)

                    def av_stage(ot=ot, exs=exs, kti=kti, m=m, qb=qb):
                        for hl in range(2):
                            h = 2 * m + hl
                            nc.tensor.matmul(
                                ot[:, hl * 512:(hl + 1) * 512],
                                lhsT=vext[:, (kti * HPC + h) * VW:(kti * HPC + h + 1) * VW],
                                rhs=exs[hl],
                                start=(kti == 0),
                                stop=(kti == KT - 1),
                            )
                        if kti == KT - 1:
                            epi_stages.extend(make_epilogue(ot, m, qb))
                    av_queue.append(av_stage)
                    emit_slot()
                    continue
                st = psp.tile([P, 1024], F32, name="st", tag="st")
                inject = "inject" in ablate
                for hl in range(2):
                    nc.tensor.matmul(
                        st[:, hl * 512:(hl + 1) * 512],
                        lhsT=kt_sb[m][hl * D:(hl + 1) * D, kti * P:(kti + 1) * P],
                        rhs=qt_sb[m][hl * D:(hl + 1) * D, qb * 512:(qb + 1) * 512],
                        start=True,
                        stop=not inject,
                    )
                if inject:
                    for hl in range(2):
                        nc.tensor.matmul(
                            st[:, hl * 512:(hl + 1) * 512],
                            lhsT=ident_bf[:, :],
                            rhs=nm[:, kti, :],
                            start=False,
                            stop=True,
                        )
                ex = workp.tile([P, 1024], BF16, name="ex", tag="ex")
                if "halfexp" in ablate:
                    nc.scalar.activation(
                        ex[:, 0:512], st[:, 0:512],
                        mybir.ActivationFunctionType.Exp, scale=1.0 / TEMP
                    )
                else:
                    nc.scalar.activation(
                        ex, st, mybir.ActivationFunctionType.Exp, scale=1.0 / TEMP
                    )
                if "nomask" not in ablate and "inject" not in ablate:
                    mask_eng = nc.gpsimd if kti < gmask_n else nc.vector
                    if "bmask" in ablate:
                        exv = ex.rearrange("p (t q) -> p t q", t=2)
                        nmb = nm[:, kti, None, :].broadcast_to([P, 2, 512])
                        mask_eng.tensor_mul(exv, exv, nmb)
                    else:
                        for hl in range(2):
                            mask_eng.tensor_mul(
                                ex[:, hl * 512:(hl + 1) * 512],
                                ex[:, hl * 512:(hl + 1) * 512],
                                nm[:, kti, :],
                            )

                def av_stage(ot=ot, ex=ex, kti=kti, m=m, qb=qb):
                    if "noav" in ablate:
                        if kti == 0:
                            nc.vector.memset(ot, 1.0)
                        if kti == KT - 1:
                            epi_stages.extend(make_epilogue(ot, m, qb))
                        return
                    for hl in range(2):
                        h = 2 * m + hl
                        nc.tensor.matmul(
                            ot[:, hl * 512:(hl + 1) * 512],
                            lhsT=vext[:, (kti * HPC + h) * VW:(kti * HPC + h + 1) * VW],
                            rhs=ex[:, hl * 512:(hl + 1) * 512],
                            start=(kti == 0),
                            stop=(kti == KT - 1),
                        )
                    if kti == KT - 1:
                        epi_stages.extend(make_epilogue(ot, m, qb))
                if "lastfast" in ablate and qb == QB - 1 and m == 1:
                    while av_queue:
                        av_queue.popleft()()
                    av_stage()
                    while epi_stages:
                        epi_stages.popleft()()
                else:
                    av_queue.append(av_stage)
                emit_slot()

    # drain pipeline
    while prelude:
        prelude.popleft()[1]()
    while av_queue:
        av_queue.popleft()()
    while epi_stages:
        epi_stages.popleft()()


_NC_CACHE = {}


def _get_program(repeat=1, ablate=()):
    key = (repeat, tuple(ablate))
    if key not in _NC_CACHE:
        _NC_CACHE[key] = _build_program(repeat, ablate=tuple(ablate))
    return _NC_CACHE[key]


def _make_in_maps(x, mask, Wq, Wk, Wv):
    in_maps = []
    for core in range(NCORES):
        b, hg = core // 2, core % 2
        hsl = slice(hg * DQ, (hg + 1) * DQ)
        in_maps.append({
            "xT": np.ascontiguousarray(x[b].T).astype(NPBF16),
            "wqT": np.ascontiguousarray(Wq[hsl, :].T).astype(NPBF16),
            "wkT": np.ascontiguousarray(Wk[hsl, :].T).astype(NPBF16),
            "wvT": np.ascontiguousarray(Wv[hsl, :].T).astype(NPBF16),
            "nmT": np.ascontiguousarray((~mask[0, b]).T).astype(NPBF16),
        })
    return in_maps


def _assemble(results):
    out = np.empty((B, N, C), dtype=np.float32)
    for core in range(NCORES):
        b, hg = core // 2, core % 2
        out[b, :, hg * DQ:(hg + 1) * DQ] = results[core]["o"]
    return out


# Default feature flags used by kernel()/run(), HW-validated 2026-08-08:
#   slack    — deeper mask/work pools + AV pipeline depth 3
#   bmask    — one broadcast mask-multiply per kti (halves DVE mask op count)
#   dmaorder — interleave x/wq/wk chunk DMAs; split mask DMA so qb starts early
#   sched2   — deadline-scheduled projection prelude (spreads PE load)
#   obatch   — batch the 8 per-slice output DMAs per (m,qb) into one
#   vconst   — hoist the vext ones-column memset out of the body (the ones
#              survive across bodies; strictly less per-body work)
# Measured (paired-slope, median of 120): baseline 122us -> 100-107us.
FLAGS = ("slack", "bmask", "dmaorder", "sched2", "obatch", "vconst")


def run(x, mask, Wq, Wk, Wv, repeat=1, ablate=None, **spmd_kwargs):
    nc = _get_program(repeat, ablate=FLAGS if ablate is None else tuple(ablate))
    in_maps = _make_in_maps(
        np.asarray(x), np.asarray(mask), np.asarray(Wq), np.asarray(Wk), np.asarray(Wv)
    )
    res = run_bass_kernel_spmd(nc, in_maps, list(range(NCORES)), **spmd_kwargs)
    return _assemble(res.results), res


def kernel(x, mask, Wq, Wk, Wv):
    out, _ = run(x, mask, Wq, Wk, Wv)
    return out

